# revision 1
# baseline (speedup 1.0000x reference)
"""BiLSTM-CRF loss kernel for 8x Trainium2 NeuronCores (Bass/Tile).

Contract: kernel(**inputs) takes the FULL unsharded inputs (numpy) and
returns the FULL scalar output, matching reference.reference().

Strategy (data-parallel over batch, 8 cores x 64 sentences):
  - emissions^T[k,(u,s,b)] = W^T @ hidden^T via bf16 PE matmuls; hidden is
    transposed on the host to (H, 256, 2, B_local): stream s=0 holds
    t = u (forward half), stream s=1 holds t = 511-u (backward half,
    time-reversed), so the contraction dim (H) lands on SBUF partitions
    with contiguous DMA and both scan directions read one 128-wide slice.
  - CRF log-partition via a product-domain BIDIRECTIONAL scan meeting at
    t=255 (Z = sum_i alpha_255[i] * beta_255[i]):
        fwd:  p <- (E'^T p) (x) e_t         (alpha, from t=0 up)
        bwd:  b <- E' (b (x) e_{t+1})       (beta, from t=511 down)
    with E' = exp(transitions) * e^{-SHIFT} in bf16. Per round: two tiny
    PE matmuls into halves of one PSUM tile + ONE 128-wide DVE multiply.
    Renormalization is off-chain: column sums are staged to the host and
    1/colsum is folded into an emissions slice two steps ahead.
  - gold-path emission gather sum_{t,b} em[t,b,tags[t,b]] via DVE
    scalar_tensor_tensor (mult+reduce) against a host-built one-hot.
  - start/end/transition gathers of the gold path are pure (tags, params)
    functions -> host scalars; final reduction in float64 on host.
"""

import os
import sys

import numpy as np

if "/opt/trn_rl_repo" not in sys.path:
    sys.path.insert(0, "/opt/trn_rl_repo")

import ml_dtypes

T, B, H, K = 512, 512, 512, 48
NCORES = 8
BL = B // NCORES          # batch per core
U = T // 2                # u-positions; u pairs (t=u, t=511-u)
UC = 16                   # u-positions per chunk
NCH = U // UC             # 8 chunks
SUB = 512                 # free elems per emissions psum tile (= 4 u-positions)
NSUB = UC * 2 * BL // SUB
SHIFT = 4.4               # e^-SHIFT folded into E' to keep the scan drift ~0
RENORM = 32
NREN_F = 7                # fwd renorms at t = 32..224
NREN_B = 7                # bwd renorms at t' = 288..480
FIN_SLOT = 14
CSLOTS = 16
ROUNDS = 255              # fwd t=1..255 ; bwd t'=510..256

_COMPILED = None
LAST_RESULT = None        # BassKernelResults of the most recent run (for test.py)


def _build(reps=1, no_scan=False, no_emissions=False, no_gather=False):
    from contextlib import ExitStack

    import concourse.tile as tile
    from concourse import bacc, mybir

    fp32 = mybir.dt.float32
    bf16 = mybir.dt.bfloat16
    AF = mybir.ActivationFunctionType
    ALU = mybir.AluOpType

    nc = bacc.Bacc(
        "TRN2", target_bir_lowering=False, debug=False, enable_asserts=False
    )
    hid = nc.dram_tensor("hidt", [H, U, 2 * BL], bf16, kind="ExternalInput").ap()
    w = nc.dram_tensor("w", [H, K], bf16, kind="ExternalInput").ap()
    ep = nc.dram_tensor("eprime", [K, K], bf16, kind="ExternalInput").ap()
    ept = nc.dram_tensor("eprimet", [K, K], bf16, kind="ExternalInput").ap()
    est = nc.dram_tensor("expstart", [K, 1], fp32, kind="ExternalInput").ap()
    een = nc.dram_tensor("expend", [K, 1], fp32, kind="ExternalInput").ap()
    bia = nc.dram_tensor("bias", [K, 1], fp32, kind="ExternalInput").ap()
    oh = nc.dram_tensor("onehot", [K, U, 2 * BL], bf16, kind="ExternalInput").ap()
    cvec = nc.dram_tensor("cvec", [1, CSLOTS * BL], fp32, kind="ExternalOutput").ap()
    emacc = nc.dram_tensor(
        "emacc", [K, NCH * NSUB], fp32, kind="ExternalOutput"
    ).ap()

    with tile.TileContext(nc) as tc:
        with ExitStack() as ctx:
            const = ctx.enter_context(tc.tile_pool(name="const", bufs=1))
            hidp = ctx.enter_context(tc.tile_pool(name="hid", bufs=12))
            ohp = ctx.enter_context(tc.tile_pool(name="oh", bufs=3))
            expp = ctx.enter_context(tc.tile_pool(name="expem", bufs=3))
            emp = ctx.enter_context(tc.tile_pool(name="embf", bufs=3))
            sttp = ctx.enter_context(tc.tile_pool(name="stt", bufs=2))
            state = ctx.enter_context(tc.tile_pool(name="state", bufs=3))
            small = ctx.enter_context(tc.tile_pool(name="small", bufs=2))
            accp = ctx.enter_context(tc.tile_pool(name="acc", bufs=1))
            pse = ctx.enter_context(tc.tile_pool(name="pse", bufs=2, space="PSUM"))
            psf = ctx.enter_context(tc.tile_pool(name="psf", bufs=2, space="PSUM"))
            psb = ctx.enter_context(tc.tile_pool(name="psb", bufs=2, space="PSUM"))
            psr = ctx.enter_context(tc.tile_pool(name="psr", bufs=2, space="PSUM"))

            # --- resident constants ---
            w_sb = const.tile([128, 4 * K], bf16)
            for hh in range(4):
                nc.sync.dma_start(
                    w_sb[:, hh * K : (hh + 1) * K], w[hh * 128 : (hh + 1) * 128, :]
                )
            ep_sb = const.tile([K, K], bf16)
            nc.sync.dma_start(ep_sb[:], ep[:])
            ept_sb = const.tile([K, K], bf16)
            nc.sync.dma_start(ept_sb[:], ept[:])
            est_sb = const.tile([K, 1], fp32)
            nc.sync.dma_start(est_sb[:], est[:])
            een_sb = const.tile([K, 1], fp32)
            nc.sync.dma_start(een_sb[:], een[:])
            bia_sb = const.tile([K, 1], fp32)
            nc.sync.dma_start(bia_sb[:], bia[:])
            ones_cb = const.tile([K, 1], bf16)
            nc.vector.memset(ones_cb[:], 1.0)
            ones_c = const.tile([K, 1], fp32)
            nc.vector.memset(ones_c[:], 1.0)
            ones_r = const.tile([1, K], fp32)
            nc.vector.memset(ones_r[:], 1.0)
            cstage = accp.tile([1, CSLOTS * BL], fp32)
            nc.gpsimd.memset(cstage[:], 1.0)
            emacc_sb = accp.tile([K, NCH * NSUB], fp32)
            if no_emissions:
                nc.gpsimd.memset(emacc_sb[:], 0.0)

            chunk_tiles = [None] * NCH
            CF = UC * 2 * BL  # free elems per chunk tile (4096)

            def emit_dmas(c):
                hts = []
                for hh in range(4):
                    ht = hidp.tile([128, CF], bf16, tag="hid", name="hid_t")
                    src_ap = hid[hh * 128 : (hh + 1) * 128, c * UC : (c + 1) * UC, :]
                    if c == 0:
                        cut = SUB // (2 * BL)  # u-cols covered by the first psum tile
                        nc.sync.dma_start(ht[:, : SUB], src_ap[:, :cut, :])
                        nc.sync.dma_start(ht[:, SUB :], src_ap[:, cut:, :])
                    else:
                        nc.sync.dma_start(ht[:], src_ap)
                    hts.append(ht)
                oht = ohp.tile([K, CF], bf16, tag="oh", name="oh_t")
                nc.sync.dma_start(oht[:], oh[:, c * UC : (c + 1) * UC, :])
                expem = expp.tile([K, CF], bf16, tag="expem", name="expem_t")
                embf = emp.tile([K, CF], bf16, tag="embf", name="embf_t")
                if no_emissions:
                    nc.gpsimd.memset(expem[:], 1.0)
                chunk_tiles[c] = (hts, oht, expem, embf)

            def emission_ops(c):
                """Generator of thunks; each emits one instruction."""
                if no_emissions:
                    return
                hts, oht, expem, embf = chunk_tiles[c]
                ps_box = [None]
                for s in range(NSUB):
                    def mk_mm(s, hh):
                        def f():
                            if hh == 0:
                                ps_box[0] = pse.tile(
                                    [K, SUB], fp32, tag="pse", name="ps_em"
                                )
                            nc.tensor.matmul(
                                ps_box[0][:],
                                w_sb[:, hh * K : (hh + 1) * K],
                                hts[hh][:, s * SUB : (s + 1) * SUB],
                                start=(hh == 0),
                                stop=(hh == 3),
                            )
                        return f
                    for hh in range(4):
                        yield mk_mm(s, hh)

                    def mk_acts(s):
                        def f():
                            ps = ps_box[0]
                            nc.scalar.activation(
                                expem[:, s * SUB : (s + 1) * SUB],
                                ps[:],
                                AF.Exp,
                                bias=bia_sb[:],
                            )
                            if not no_gather:
                                nc.scalar.copy(
                                    embf[:, s * SUB : (s + 1) * SUB], ps[:]
                                )
                        return f
                    yield mk_acts(s)

                    def mk_gather(s):
                        def f():
                            if no_gather:
                                return
                            stt = sttp.tile([K, SUB], bf16, tag="stt", name="stt")
                            nc.vector.scalar_tensor_tensor(
                                stt[:],
                                embf[:, s * SUB : (s + 1) * SUB],
                                1.0,
                                oht[:, s * SUB : (s + 1) * SUB],
                                ALU.mult,
                                ALU.mult,
                                accum_out=emacc_sb[:, c * NSUB + s : c * NSUB + s + 1],
                            )
                        return f
                    yield mk_gather(s)

            def e_slice(u):
                """Combined (48, 128) slice: [:, :64] = e_u ; [:, 64:] = e_{511-u}."""
                expem = chunk_tiles[u // UC][2]
                ul = u % UC
                return expem[:, ul * 2 * BL : (ul + 1) * 2 * BL]

            prescaled = {}

            def e_used(u):
                return prescaled.pop(u) if u in prescaled else e_slice(u)

            def side_renorm(pq, slot, target_u, half):
                """Stage colsum(pq) to cstage[slot]; fold 1/colsum into the
                `half` (0=fwd cols :64, 1=bwd cols 64:) of the combined
                emissions slice consumed at round using u=target_u. Entirely
                off the serial scan chain."""
                cs = psr.tile([K, BL], fp32, tag="psr", name="cs_r")
                nc.tensor.matmul(
                    cs[:1, :], ones_cb[:], pq[:], start=True, stop=True
                )
                nc.scalar.copy(cstage[:, slot * BL : (slot + 1) * BL], cs[:1, :])
                rec = small.tile([1, BL], fp32, tag="rec", name="rec")
                nc.vector.reciprocal(rec[:], cs[:1, :])
                bc = psr.tile([K, BL], fp32, tag="psr", name="bc_r")
                nc.tensor.matmul(bc[:], ones_r[:], rec[:], start=True, stop=True)
                src = e_slice(target_u)
                e2 = state.tile([K, 2 * BL], bf16, tag="e2", name="e_rn")
                lo, hi = (0, BL) if half == 0 else (BL, 2 * BL)
                olo, ohi = (BL, 2 * BL) if half == 0 else (0, BL)
                nc.vector.tensor_mul(e2[:, lo:hi], bc[:], src[:, lo:hi])
                nc.vector.tensor_copy(e2[:, olo:ohi], src[:, olo:ohi])
                prescaled[target_u] = e2

            p = None
            for rep in range(reps):
                emit_dmas(0)
                for op in emission_ops(0):
                    op()

                pending = []
                if not no_scan:
                    # init: p_0 = e_0 (x) exp(start);  q_511 = e_511 (x) exp(end)
                    e0 = e_slice(0)
                    p = state.tile([K, BL], bf16, tag="p", name="p_init")
                    nc.vector.tensor_scalar_mul(p[:], e0[:, :BL], est_sb[:])
                    q = state.tile([K, BL], bf16, tag="q", name="q_init")
                    nc.vector.tensor_scalar_mul(q[:], e0[:, BL:], een_sb[:])
                    if rep > 0:
                        # value-preserving dep on the previous rep's final
                        # output so multi-rep timing builds execute serially:
                        # p <- (bcast(prev fin) * 0) + p
                        bcf = psr.tile([K, BL], fp32, tag="psr", name="bcf")
                        nc.tensor.matmul(
                            bcf[:], ones_r[:],
                            cstage[:, FIN_SLOT * BL : (FIN_SLOT + 1) * BL],
                            start=True, stop=True,
                        )
                        p2i = state.tile([K, BL], bf16, tag="p", name="p_ser")
                        nc.vector.scalar_tensor_tensor(
                            p2i[:], bcf[:], 0.0, p[:], ALU.mult, ALU.add
                        )
                        p = p2i
                        q2i = state.tile([K, BL], bf16, tag="q", name="q_ser")
                        nc.vector.scalar_tensor_tensor(
                            q2i[:], bcf[:], 0.0, q[:], ALU.mult, ALU.add
                        )
                        q = q2i

                    nfwd_r = 0
                    nbwd_r = 0
                    for r in range(ROUNDS):
                        if r % UC == 0 and r // UC < NCH - 1:
                            emit_dmas(r // UC + 1)
                            pending += list(emission_ops(r // UC + 1))
                        per_step = (max(1, (len(pending) + UC - 1) // UC)
                                    if pending else 0)

                        t = r + 1          # fwd produces p_t
                        tb = 510 - r       # bwd produces b_tb then q_tb
                        eu = e_used(t)     # (48, 128): [e_t | e_tb]
                        sf = psf.tile([K, BL], fp32, tag="psf", name="ps_f")
                        nc.tensor.matmul(sf[:], ep_sb[:], p[:], start=True, stop=True)
                        p2 = state.tile([K, BL], bf16, tag="p", name="p_s")
                        nc.vector.tensor_mul(p2[:], sf[:], eu[:, :BL])
                        p = p2
                        sb = psb.tile([K, BL], fp32, tag="psb", name="ps_b")
                        nc.tensor.matmul(sb[:], ept_sb[:], q[:], start=True, stop=True)
                        q2 = state.tile([K, BL], bf16, tag="q", name="q_s")
                        nc.vector.tensor_mul(q2[:], sb[:], eu[:, BL:])
                        q = q2

                        if t % RENORM == 0 and t // RENORM <= NREN_F:
                            side_renorm(p, t // RENORM - 1, t + 2, 0)
                            nfwd_r += 1
                        if tb % RENORM == 0 and 9 <= tb // RENORM <= 15:
                            side_renorm(q, NREN_F + (15 - tb // RENORM),
                                        513 - tb, 1)
                            nbwd_r += 1

                        for _ in range(per_step):
                            if pending:
                                pending.pop(0)()
                    while pending:
                        pending.pop(0)()
                    assert nfwd_r == NREN_F and nbwd_r == NREN_B, (nfwd_r, nbwd_r)
                    assert not prescaled, list(prescaled)

                    # meeting at t=255: Z = sum_i p_255[i] * b_255[i]
                    b255 = psr.tile([K, BL], fp32, tag="psr", name="ps_b255")
                    nc.tensor.matmul(b255[:], ept_sb[:], q[:], start=True, stop=True)
                    m = state.tile([K, BL], fp32, tag="m", name="meet")
                    nc.vector.tensor_mul(m[:], b255[:], p[:])
                    fin = psr.tile([K, BL], fp32, tag="psr", name="fin")
                    nc.tensor.matmul(
                        fin[:1, :], ones_c[:], m[:], start=True, stop=True
                    )
                    nc.scalar.copy(
                        cstage[:, FIN_SLOT * BL : (FIN_SLOT + 1) * BL], fin[:1, :]
                    )
                else:
                    for c in range(1, NCH):
                        emit_dmas(c)
                        for op in emission_ops(c):
                            op()

                # inside the rep loop: on the HWDGE FIFO these gate the next
                # rep's input DMAs, serializing reps for latency measurement
                nc.sync.dma_start(cvec[:], cstage[:])
                nc.sync.dma_start(emacc[:], emacc_sb[:])

    nc.compile()
    return nc


def _get_compiled():
    global _COMPILED
    if _COMPILED is None:
        _COMPILED = _build()
    return _COMPILED


def _numpy_reference(hidden, W, b, start_transitions, end_transitions, transitions,
                     tags, mask):
    """Plain numpy fallback (only used if mask is not all ones)."""
    em = hidden.astype(np.float64) @ W.astype(np.float64) + b.astype(np.float64)
    maskf = mask.astype(np.float64)
    bar = np.arange(em.shape[1])
    st = start_transitions.astype(np.float64)
    en = end_transitions.astype(np.float64)
    tr = transitions.astype(np.float64)
    num = st[tags[0]] + em[0, bar, tags[0]]
    trs = tr[tags[:-1], tags[1:]]
    ems = np.take_along_axis(em[1:], tags[1:][..., None], axis=2)[..., 0]
    num = num + ((trs + ems) * maskf[1:]).sum(axis=0)
    seq_ends = mask.astype(np.int64).sum(axis=0) - 1
    num = num + en[tags[seq_ends, bar]]
    score = st[None, :] + em[0]
    for t in range(1, em.shape[0]):
        nxt = score[:, :, None] + tr[None] + em[t][:, None, :]
        m = nxt.max(axis=1)
        nxt = m + np.log(np.exp(nxt - m[:, None, :]).sum(axis=1))
        score = np.where(mask[t][:, None], nxt, score)
    fm = score + en[None, :]
    mm = fm.max(axis=1)
    denom = mm + np.log(np.exp(fm - mm[:, None]).sum(axis=1))
    return np.float32((num - denom).sum())


def kernel(hidden, W, b, start_transitions, end_transitions, transitions, tags,
           mask):
    hidden = np.asarray(hidden)
    W = np.asarray(W, dtype=np.float32)
    b = np.asarray(b, dtype=np.float32)
    start_transitions = np.asarray(start_transitions, dtype=np.float32)
    end_transitions = np.asarray(end_transitions, dtype=np.float32)
    transitions = np.asarray(transitions, dtype=np.float32)
    tags = np.asarray(tags)
    mask = np.asarray(mask)

    if not mask.all():
        return _numpy_reference(hidden, W, b, start_transitions, end_transitions,
                                transitions, tags, mask)

    from concourse.bass_utils import run_bass_kernel_spmd

    nc = _get_compiled()
    in_maps = _prepare_in_maps(hidden, W, b, start_transitions, end_transitions,
                               transitions, tags)

    global LAST_RESULT
    res = run_bass_kernel_spmd(nc, in_maps, core_ids=list(range(NCORES)))
    LAST_RESULT = res

    return _host_reduce(b, start_transitions, end_transitions, transitions, tags,
                        res.results)


def _fold_bidir(arr_t_last):
    """(X, T, BL) -> (X, U, 2, BL) with stream 1 time-reversed."""
    x, t, bl = arr_t_last.shape
    out = np.empty((x, U, 2, bl), dtype=arr_t_last.dtype)
    out[:, :, 0, :] = arr_t_last[:, :U, :]
    out[:, :, 1, :] = arr_t_last[:, : U - 1 : -1, :]
    return out.reshape(x, U, 2 * bl)


def _prepare_in_maps(hidden, W, b, start_transitions, end_transitions,
                     transitions, tags):
    w_bf = W.astype(ml_dtypes.bfloat16)
    eprime64 = np.exp(transitions.astype(np.float64)) * np.exp(-SHIFT)
    eprime = eprime64.astype(ml_dtypes.bfloat16)
    eprimet = np.ascontiguousarray(eprime64.T).astype(ml_dtypes.bfloat16)
    expstart = np.exp(start_transitions).reshape(K, 1).astype(np.float32)
    expend = np.exp(end_transitions).reshape(K, 1).astype(np.float32)
    bias = b.reshape(K, 1).astype(np.float32)
    onehot = (
        (tags[None, :, :] == np.arange(K, dtype=tags.dtype)[:, None, None])
        .astype(ml_dtypes.bfloat16)
    )  # (K, T, B)

    in_maps = []
    for c in range(NCORES):
        sl = slice(c * BL, (c + 1) * BL)
        hidt = hidden[:, sl, :].transpose(2, 0, 1).astype(ml_dtypes.bfloat16)
        in_maps.append(
            {
                "hidt": np.ascontiguousarray(_fold_bidir(hidt)),
                "w": w_bf,
                "eprime": eprime,
                "eprimet": eprimet,
                "expstart": expstart,
                "expend": expend,
                "bias": bias,
                "onehot": np.ascontiguousarray(_fold_bidir(onehot[:, :, sl])),
            }
        )
    return in_maps


def _host_reduce(b, start_transitions, end_transitions, transitions, tags,
                 results):
    tagsl = tags.astype(np.int64)
    total = np.float64(0.0)
    total += start_transitions.astype(np.float64)[tagsl[0]].sum()
    total += transitions.astype(np.float64)[tagsl[:-1], tagsl[1:]].sum()
    total += end_transitions.astype(np.float64)[tagsl[-1]].sum()
    total += b.astype(np.float64)[tagsl].sum()  # bias part of the em gather

    for c in range(NCORES):
        out = results[c]
        total += out["emacc"].astype(np.float64).sum()
        cv = out["cvec"].astype(np.float64).reshape(CSLOTS, BL)
        denom_b = (
            np.log(cv[: NREN_F + NREN_B]).sum(axis=0)
            + np.log(cv[FIN_SLOT])
            + (T - 1) * SHIFT
        )
        total -= denom_b.sum()

    return np.float32(total)

